# revision 18
# baseline (speedup 1.0000x reference)
"""MoE routing kernel for Trainium2 (8 NeuronCores, Bass/Tile).

Strategy: the reference computes all 8 experts densely on all 4096 tokens, but
only the top-2 experts per token survive the combine. Since sum(topk_w) == 1,
    final = x*(1+rs) + ss*sigmoid(x@Ws+bs) + rs*sum_k w_k*sigmoid(x@W_ek+b_ek)
so every needed matmul row has the unified form  w_slot * sigmoid(x_t @ W_m + b_m).
We flatten all (token, matrix) pairs (2 expert slots/token + 1 shared slot/token),
group them into 128-token chunks per matrix, and bin-pack the chunks evenly
across 8 cores x NSEG weight segments. Each core runs the same program (SPMD)
on its own gathered tokens + segment weight matrices. The gating network,
top-k selection, gather/scatter and the final residual add run on host.
"""

import sys

sys.path.insert(0, "/opt/trn_rl_repo")

import numpy as np

HIDDEN = 2048
N_EXPERTS = 8
GATING_DIM = 64
TOP_K = 2
BALANCING_COEF = 0.01
NOISE_STD = 0.01
N_CORES = 8
P = 128
KO = HIDDEN // P  # 16 contraction subtiles
NSEG = 3  # weight matrices per core
NBLK = 4  # feature-dim blocks
FB = HIDDEN // NBLK  # 512

_PROG_CACHE: dict = {}


import os

MM_DTYPE = os.environ.get("MOE_MM_DTYPE", "float32r")  # "float32r" | "bfloat16"


def _build_program(CH, seg_sizes, has_bias, reps=1, mm_dtype=None):
    from concourse import bacc
    import concourse.tile as tile
    import concourse.mybir as mybir

    C = CH * P
    nc = bacc.Bacc("TRN2", target_bir_lowering=False, debug=False, num_devices=N_CORES)
    # float32r: full 4-byte fp32 storage, PE streams it at 1 cycle/row for
    # moving dims >= 256 (vs 4 cycles/row for plain fp32) at ~tf32 precision.
    mm_dt = getattr(mybir.dt, mm_dtype or MM_DTYPE)
    xgt = nc.dram_tensor("xgt", [HIDDEN, C], mm_dt, kind="ExternalInput")
    wseg = nc.dram_tensor(
        "wseg", [NSEG, HIDDEN, HIDDEN], mm_dt, kind="ExternalInput"
    )
    wtok = nc.dram_tensor("wtok", [C], mybir.dt.float32, kind="ExternalInput")
    if has_bias:
        bseg = nc.dram_tensor(
            "bseg", [NSEG, P, HIDDEN], mybir.dt.float32, kind="ExternalInput"
        )
    contrib = nc.dram_tensor(
        "contrib", [C, HIDDEN], mybir.dt.float32, kind="ExternalOutput"
    )

    KOQ = 4  # contraction subtiles per weight-block quarter (1 MiB DMAs)
    NQ = KO // KOQ
    # xg slots: current segment + one segment of prefetch; weight slots:
    # 3 blocks in flight. Keeps per-partition SBUF below the 192 KiB cap.
    xg_bufs = min(CH, max(seg_sizes) + (max(seg_sizes[1:]) if NSEG > 1 else 0))
    xgt3 = xgt.rearrange("(ko p) c -> p ko c", p=P)
    with tile.TileContext(nc) as tc:
        with tc.tile_pool(name="resident", bufs=1) as resident, tc.tile_pool(
            name="xgpool", bufs=xg_bufs
        ) as xgpool, tc.tile_pool(name="wpool", bufs=3 * NQ) as wpool, tc.tile_pool(
            name="opool", bufs=4
        ) as opool, tc.tile_pool(name="pspool", bufs=4, space="PSUM") as pspool:
            wtok_t = resident.tile([P, CH], mybir.dt.float32)
            nc.sync.dma_start(wtok_t, wtok.rearrange("(c p) -> p c", p=P))
            if has_bias:
                bseg_t = resident.tile([P, NSEG, HIDDEN], mybir.dt.float32)
                nc.sync.dma_start(bseg_t, bseg.rearrange("s p h -> p s h"))

            # Per-chunk gathered-token tiles; DMAs are emitted lazily at the
            # start of the owning segment so the first matmuls only wait on
            # their own 1 MiB slice instead of the whole gather.
            xg_tiles = [None] * CH

            def load_chunk(ch):
                t = xgpool.tile([P, KO, P], mm_dt, tag="xg")
                nc.sync.dma_start(t, xgt3[:, :, ch * P : (ch + 1) * P])
                xg_tiles[ch] = t

            for _rep in range(reps):
                _emit_body(nc, tc, seg_sizes, load_chunk, xg_tiles, wpool, opool,
                           pspool, wtok_t, bseg_t if has_bias else None, wseg,
                           contrib, mm_dt, has_bias)
    nc.compile()
    return nc


def _emit_body(nc, tc, seg_sizes, load_chunk, xg_tiles, wpool, opool, pspool,
               wtok_t, bseg_t, wseg, contrib, mm_dt, has_bias):
    import concourse.mybir as mybir

    KOQ = 4
    NQ = KO // KOQ
    if True:
            cbase = 0
            for s in range(NSEG):
                for mc in range(seg_sizes[s]):
                    load_chunk(cbase + mc)
                for nb in range(NBLK):
                    w_ts = []
                    for q in range(NQ):
                        wq = wpool.tile([P, KOQ, FB], mm_dt, tag="w")
                        nc.sync.dma_start(
                            wq,
                            wseg[
                                s,
                                q * KOQ * P : (q + 1) * KOQ * P,
                                nb * FB : (nb + 1) * FB,
                            ].rearrange("(ko p) f -> p ko f", p=P),
                        )
                        w_ts.append(wq)
                    for mc in range(seg_sizes[s]):
                        ch = cbase + mc
                        ps = pspool.tile([P, FB], mybir.dt.float32)
                        for k in range(KO):
                            nc.tensor.matmul(
                                ps,
                                xg_tiles[ch][:, k, :],
                                w_ts[k // KOQ][:, k % KOQ, :],
                                start=(k == 0),
                                stop=(k == KO - 1),
                            )
                        ot = opool.tile([P, FB], mybir.dt.float32, tag="o")
                        if has_bias:
                            nc.vector.tensor_tensor(
                                ot,
                                ps,
                                bseg_t[:, s, nb * FB : (nb + 1) * FB],
                                mybir.AluOpType.add,
                            )
                            nc.scalar.activation(
                                ot, ot, mybir.ActivationFunctionType.Sigmoid
                            )
                        else:
                            nc.scalar.activation(
                                ot, ps, mybir.ActivationFunctionType.Sigmoid
                            )
                        nc.vector.tensor_scalar_mul(ot, ot, wtok_t[:, ch : ch + 1])
                        nc.sync.dma_start(
                            contrib[ch * P : (ch + 1) * P, nb * FB : (nb + 1) * FB], ot
                        )
                cbase += seg_sizes[s]


def _get_program(CH, seg_sizes, has_bias, reps=1, mm_dtype=None):
    key = (CH, tuple(seg_sizes), has_bias, reps, mm_dtype or MM_DTYPE)
    if key not in _PROG_CACHE:
        _PROG_CACHE[key] = _build_program(
            CH, seg_sizes, has_bias, reps=reps, mm_dtype=mm_dtype
        )
    return _PROG_CACHE[key]


def _seg_split(CH):
    base = CH // NSEG
    rem = CH % NSEG
    return [base + (1 if i < rem else 0) for i in range(NSEG)]


def _pack(chunk_counts, CH):
    """Assign each matrix's chunks to (core, seg) pieces.

    chunk_counts: list of (matrix_id, n_chunks), n_chunks > 0.
    Pieces: N_CORES copies of each entry in _seg_split(CH). Returns
    list of (matrix_id, piece_size, used_chunks) per piece, or None.
    """
    seg_sizes = _seg_split(CH)
    sizes = sorted(set(seg_sizes), reverse=True)
    avail = {sz: seg_sizes.count(sz) * N_CORES for sz in sizes}
    total_cap = sum(sz * n for sz, n in avail.items())
    total_need = sum(n for _, n in chunk_counts)
    slack = total_cap - total_need
    if slack < 0:
        return None
    order = sorted(chunk_counts, key=lambda t: -t[1])
    nodes = [0]

    def combos(need, avail, budget):
        """All piece-count dicts covering `need` with waste <= budget."""
        out = []

        def rec(i, rem, cur):
            if rem <= 0:
                out.append((dict(cur), -rem))
                return
            if i == len(sizes):
                return
            sz = sizes[i]
            maxk = min(avail[sz], (rem + budget) // sz + 1)
            for k in range(maxk, -1, -1):
                if sz * k - rem > budget:
                    continue
                cur[sz] = k
                rec(i + 1, rem - sz * k, cur)
            cur.pop(sz, None)

        rec(0, need, {})
        out.sort(key=lambda t: t[1])
        return out

    def dfs(i, avail, budget):
        nodes[0] += 1
        if nodes[0] > 20000:
            return None
        if i == len(order):
            return []
        m, need = order[i]
        for cnt, waste in combos(need, avail, budget):
            nav = dict(avail)
            ok = True
            for sz, k in cnt.items():
                nav[sz] -= k
                if nav[sz] < 0:
                    ok = False
            if not ok:
                continue
            rest = dfs(i + 1, nav, budget - waste)
            if rest is not None:
                pieces = []
                rem = need
                for sz in sizes:
                    for _ in range(cnt.get(sz, 0)):
                        used = min(sz, rem)
                        pieces.append((m, sz, used))
                        rem -= used
                return pieces + rest
        return None

    return dfs(0, avail, slack)


def _route_and_pack(x, gating_np, topk_idx, topk_w, router_scale, shared_scale):
    """Build per-core device inputs + the inverse maps for reassembly."""
    B = x.shape[0]
    rs = float(router_scale)
    ss = float(shared_scale)

    # Token lists per matrix: experts 0..7, shared = 8.
    tok_lists = []
    wt_lists = []
    for m in range(N_EXPERTS):
        sel0 = np.nonzero(topk_idx[:, 0] == m)[0]
        sel1 = np.nonzero(topk_idx[:, 1] == m)[0]
        toks = np.concatenate([sel0, sel1]).astype(np.int64)
        wts = np.concatenate([topk_w[sel0, 0], topk_w[sel1, 1]]).astype(np.float32) * rs
        tok_lists.append(toks)
        wt_lists.append(wts)
    tok_lists.append(np.arange(B, dtype=np.int64))
    wt_lists.append(np.full(B, ss, dtype=np.float32))

    chunk_counts = [
        (m, (len(tok_lists[m]) + P - 1) // P)
        for m in range(N_EXPERTS + 1)
        if len(tok_lists[m]) > 0
    ]
    n_chunks_total = sum(n for _, n in chunk_counts)
    CH = max(1, (n_chunks_total + N_CORES - 1) // N_CORES)
    pieces = None
    while pieces is None:
        pieces = _pack(chunk_counts, CH)
        if pieces is None:
            CH += 1
    seg_sizes = _seg_split(CH)
    C = CH * P

    # Distribute pieces onto concrete (core, seg) slots of matching size.
    slots_by_size = {}
    for core in range(N_CORES):
        for s, sz in enumerate(seg_sizes):
            slots_by_size.setdefault(sz, []).append((core, s))
    placement = {}  # (core, seg) -> (matrix, chunk_lo, used_chunks)
    consumed = {m: 0 for m, _ in chunk_counts}
    for m, sz, used in pieces:
        core, s = slots_by_size[sz].pop()
        placement[(core, s)] = (m, consumed[m], used)
        consumed[m] += used

    # Per-core arrays + global slot map per matrix.
    xgt_list, wtok_list, wseg_list, bmat_list = [], [], [], []
    slot_of = {m: np.zeros(len(tok_lists[m]), dtype=np.int64) for m, _ in chunk_counts}
    for core in range(N_CORES):
        tok = np.zeros(C, dtype=np.int64)
        wt = np.zeros(C, dtype=np.float32)
        mats = np.full(NSEG, N_EXPERTS, dtype=np.int64)  # default: shared weights
        cbase = 0
        for s in range(NSEG):
            if (core, s) in placement:
                m, lo, used = placement[(core, s)]
                mats[s] = m
                p0 = lo * P
                p1 = min(p0 + used * P, len(tok_lists[m]))
                n = p1 - p0
                base = cbase * P
                tok[base : base + n] = tok_lists[m][p0:p1]
                wt[base : base + n] = wt_lists[m][p0:p1]
                slot_of[m][p0:p1] = core * C + base + np.arange(n)
            cbase += seg_sizes[s]
        xg = x[tok]  # [C, H]
        xgt_list.append(np.ascontiguousarray(xg.T))
        wtok_list.append(wt)
        wseg_list.append(mats)
        bmat_list.append(mats)

    return {
        "CH": CH,
        "seg_sizes": seg_sizes,
        "C": C,
        "xgt": xgt_list,
        "wtok": wtok_list,
        "mats": wseg_list,
        "tok_lists": tok_lists,
        "slot_of": slot_of,
    }


def _gating(x, noise, gate_w1, gate_b1, gate_w2, gate_b2):
    """Mirror the reference gating ops exactly (same jax ops, default backend)
    so top-k selection matches the reference bit-for-bit."""
    import jax
    import jax.numpy as jnp

    xj = jnp.asarray(x)
    h = jax.nn.relu(xj @ jnp.asarray(gate_w1) + jnp.asarray(gate_b1))
    gating = jax.nn.softmax(h @ jnp.asarray(gate_w2) + jnp.asarray(gate_b2), axis=-1)
    gating = gating + jnp.asarray(noise) * NOISE_STD
    topk_w, topk_idx = jax.lax.top_k(gating, TOP_K)
    topk_w = topk_w / jnp.sum(topk_w, axis=-1, keepdims=True)

    experts_prob = jnp.mean(gating, axis=0)
    target_prob = jnp.ones((N_EXPERTS,), xj.dtype) / N_EXPERTS
    gate_loss = jnp.mean((target_prob - experts_prob) ** 2) * BALANCING_COEF
    return (
        np.asarray(gating),
        np.asarray(topk_idx),
        np.asarray(topk_w),
        np.asarray(gate_loss),
    )


def _run(inputs, trace=False):
    from concourse.bass_utils import run_bass_kernel_spmd

    x = np.asarray(inputs["x"], dtype=np.float32)
    noise = np.asarray(inputs["noise"], dtype=np.float32)
    expert_w = np.asarray(inputs["expert_w"], dtype=np.float32)
    expert_b = np.asarray(inputs["expert_b"], dtype=np.float32)
    shared_w = np.asarray(inputs["shared_w"], dtype=np.float32)
    shared_b = np.asarray(inputs["shared_b"], dtype=np.float32)
    rs = float(np.asarray(inputs["router_scale"]))
    ss = float(np.asarray(inputs["shared_scale"]))

    gating, topk_idx, topk_w, gate_loss = _gating(
        x,
        noise,
        inputs["gate_w1"],
        inputs["gate_b1"],
        inputs["gate_w2"],
        inputs["gate_b2"],
    )

    rp = _route_and_pack(x, gating, topk_idx, topk_w, rs, ss)
    CH, seg_sizes, C = rp["CH"], rp["seg_sizes"], rp["C"]

    # Weight matrices per matrix id (expert_w layout [E, L=1, H_in, H_out]).
    if MM_DTYPE == "bfloat16":
        import ml_dtypes

        mm_np = ml_dtypes.bfloat16
    else:
        mm_np = np.float32
    w_of = [
        np.ascontiguousarray(expert_w[m, 0]).astype(mm_np, copy=False)
        for m in range(N_EXPERTS)
    ]
    w_of.append(np.ascontiguousarray(shared_w).astype(mm_np, copy=False))
    b_of = [expert_b[m, 0] for m in range(N_EXPERTS)]
    b_of.append(shared_b)
    has_bias = bool(np.any(expert_b) or np.any(shared_b))

    nc = _get_program(CH, seg_sizes, has_bias)

    in_maps = []
    for core in range(N_CORES):
        mats = rp["mats"][core]
        wseg = np.stack([w_of[m] for m in mats])
        im = {
            "xgt": rp["xgt"][core].astype(mm_np, copy=False),
            "wseg": wseg,
            "wtok": rp["wtok"][core],
        }
        if has_bias:
            im["bseg"] = np.stack(
                [np.broadcast_to(b_of[m], (P, HIDDEN)) for m in mats]
            ).astype(np.float32)
        in_maps.append(im)

    res = run_bass_kernel_spmd(
        nc, in_maps, core_ids=list(range(N_CORES)), trace=trace
    )

    contrib = np.concatenate([res.results[c]["contrib"] for c in range(N_CORES)], axis=0)

    out = x * np.float32(1.0 + rs)
    for m, _ in [(m, None) for m in range(N_EXPERTS + 1)]:
        toks = rp["tok_lists"][m]
        if len(toks) == 0:
            continue
        slots = rp["slot_of"].get(m)
        if slots is None:
            continue
        out[toks] += contrib[slots]

    return out.astype(np.float32), np.float32(gate_loss), res


def kernel(**inputs):
    out, gate_loss, _ = _run(inputs, trace=False)
    return out, gate_loss
